# revision 6
# baseline (speedup 1.0000x reference)
"""Trainium2 Bass kernel for nn_ContrastiveLossWithAttention.

Contract: kernel(**inputs) takes the FULL unsharded inputs (as produced by
reference.setup_inputs) and returns the FULL output (a float32 scalar).

Sharding: pure data parallel — batch dim B=16 split as 2 batches per core
across 8 NeuronCores. Host applies the O(B*N) scalar epilogue and the final
scalar reduction across cores.

Algorithm (gt_perm is an identity permutation restricted to rows i < src_ns,
verified exactly host-side with a numpy fallback): the loss collapses to two
masked threshold reductions over pred alone:
    T1row[i] = sum_j 1{p_ij >= t_i} p_ij^2      t_i = clip(diag_i) - beta
    T1col[j] = sum_i 1{p_ij >= tau_j} p_ij^2    tau_j = same vector, j-indexed
    corr     = sum_{j<s} (T1col_j - srcpos_j)
    loss_b   = -0.5 sum_{i<s} [ln(srcpos_i) - ln(1 + T1row_i - srcpos_i + corr)]

Since p >= 0 and thresholds are clamped at 0, p >= t  <=>  p^2 >= max(t,0)^2.
The host ships q = fp8_e4m3(clip(p)^2) (same role as the baseline's bf16 cast,
half the bytes) plus per-row/per-col squared thresholds. The device then needs
a single fused custom-DVE pass per 128-row chunk (registered at import as
CONTRA_FUSED_ANT, 1x mode):
    out[k]  = (q[k] >= tau2[k]) q[k]  +  (tau2[k] < 0) * cumsum((q[k] >= t2) q[k])
With tau2 = -128 for j >= 1536 (columns the epilogue never reads; src_ns <=
1536) and q[:, 2047] host-zeroed, out[:, :1536] is the column product that PE
ones-matmuls accumulate into T1col, and out[:, 2047] is the full row sum
T1row for the chunk (scalar-engine copy harvests it). One DVE instruction per
chunk replaces the baseline's STT + 2 TT + ACT-square pipeline.
"""

import numpy as np
import ml_dtypes

B, N, M = 16, 2048, 2048
NCORES = 8
BPC = B // NCORES      # batches per core
PT = 128               # partitions
CHR = 12               # row chunks computed: src_ns <= 1536 always
                       # (setup_inputs range); guarded with a fallback
NR = PT * CHR          # rows computed on device
NCOL = 1536            # columns with real col-thresholds (j < src max)
NQ = NCOL // 512       # 512-wide PSUM quadrants for PE column sums
SENT = -128.0          # sentinel tau2 for j >= NCOL (negative => gate on)

_cache = {}


def _register_op():
    import concourse.dve_ops as dve_ops
    from concourse.dve_spec import Spec, Src0, Src1, C0, Zero, AluOp, scan, lower
    from concourse.dve_uop import DveOpSpec
    from concourse.dve_table_gen import dve_ver_for

    for op in dve_ops.OPS:
        if op.name == "CONTRA_FUSED_ANT":
            return op

    cmp_c = Src0 >= Src1
    mul_c = cmp_c * Src0
    cmp_r = Src0 >= C0
    mul_r = cmp_r * Src0
    R = scan(AluOp.ADD, mul_r)
    gate = Src1 < Zero
    body = mul_c + gate * R

    def _ref(in0, in1, c0, c1, c2):
        f = in0.astype(np.float32)
        t = in1.astype(np.float32)
        colp = (f >= t).astype(np.float32) * f
        rowp = (f >= c0).astype(np.float32) * f
        Rv = np.cumsum(rowp, axis=-1, dtype=np.float32)
        return colp + (t < 0).astype(np.float32) * Rv

    spec = Spec(body=body, reference=_ref)
    ver = dve_ver_for("TRN2")
    row = max(dve_ops._SUB_OPCODE_FOR_NAME.values()) + 1
    sha = DveOpSpec(name="CONTRA_FUSED_ANT", opcode=row,
                    uops=lower(spec, ver=ver), rd1_en=True).sha(ver)
    op = dve_ops.DveOp("CONTRA_FUSED_ANT", spec, subdim=False,
                       uops_sha={ver: sha})
    dve_ops.OPS.append(op)
    dve_ops.CUSTOM_DVE_SPECS[op.name] = op.spec
    dve_ops._SUB_OPCODE_FOR_NAME[op.name] = row
    return op


def _build_program():
    import concourse.tile as tile
    from concourse import bacc, mybir

    OP = _register_op()
    f32 = mybir.dt.float32
    bf16 = mybir.dt.bfloat16
    fp8 = mybir.dt.float8e4

    nc = bacc.Bacc("TRN2", debug=False, num_devices=NCORES)

    q_d = nc.dram_tensor("q8", [BPC, NR, M], fp8, kind="ExternalInput")
    thr_d = nc.dram_tensor("thr2", [BPC, NR], f32, kind="ExternalInput")
    tau_d = nc.dram_tensor("tau2", [BPC, M], fp8, kind="ExternalInput")
    # t1row is [p, k] (partition-major) so the output DMA is contiguous per
    # partition; the host untransposes. A "(k p)" layout makes the DMA emit
    # 1536 4-byte descriptors (~10us exposed tail).
    t1r_d = nc.dram_tensor("t1row", [BPC, PT, CHR], f32, kind="ExternalOutput")
    t1c_d = nc.dram_tensor("t1col", [BPC, NCOL], f32, kind="ExternalOutput")

    with tile.TileContext(nc) as tc:
        with (
            tc.tile_pool(name="consts", bufs=1) as consts,
            tc.tile_pool(name="pb", bufs=2) as pb,
            tc.tile_pool(name="io", bufs=6) as io,
            tc.tile_pool(name="work", bufs=8) as work,
            tc.tile_pool(name="ps_col", bufs=2, space="PSUM") as ps_col,
        ):
            ones16 = consts.tile([PT, 1], bf16, tag="ones16")
            nc.vector.memset(ones16, 1.0)

            # prefetch all per-batch vectors before the chunk loops so the
            # batch-1 thresholds are ready the moment batch 0 drains
            thr2s, tau2s = [], []
            for b in range(BPC):
                thr2 = pb.tile([PT, CHR], f32, tag="thr2")
                nc.sync.dma_start(out=thr2, in_=thr_d[b].rearrange("(k p) -> p k", p=PT))
                tau2 = pb.tile([PT, M], fp8, tag="tau2")
                nc.sync.dma_start(
                    out=tau2, in_=tau_d[b:b + 1, :].to_broadcast([PT, M])
                )
                thr2s.append(thr2)
                tau2s.append(tau2)

            for b in range(BPC):
                thr2, tau2 = thr2s[b], tau2s[b]
                t1c_ps = ps_col.tile([1, NCOL], f32, tag="t1col")
                t1row = pb.tile([PT, CHR], f32, tag="t1row")
                for k in range(CHR):
                    q = io.tile([PT, M], fp8, tag="q")
                    nc.sync.dma_start(out=q, in_=q_d[b, k * PT:(k + 1) * PT, :])
                    oc = work.tile([PT, M], bf16, tag="oc")
                    nc.vector._custom_dve(
                        OP, out=oc, in0=q, in1=tau2,
                        s0=thr2[:, k:k + 1], s1=0.0,
                    )
                    nc.scalar.copy(t1row[:, k:k + 1], oc[:, M - 1:M])
                    for qd in range(NQ):
                        nc.tensor.matmul(
                            t1c_ps[0:1, qd * 512:(qd + 1) * 512],
                            ones16,
                            oc[:, qd * 512:(qd + 1) * 512],
                            start=(k == 0), stop=(k == CHR - 1),
                        )

                t1c_row = pb.tile([1, NCOL], f32, tag="t1c_row")
                nc.scalar.copy(t1c_row, t1c_ps[0:1, :])
                nc.sync.dma_start(out=t1c_d[b:b + 1, :], in_=t1c_row)
                nc.sync.dma_start(out=t1r_d[b], in_=t1row)

    nc.compile()
    return nc


def _get_program():
    if "nc" not in _cache:
        _cache["nc"] = _build_program()
    return _cache["nc"]


def _gt_is_identity_perm(gt_perm, src_ns):
    """Exact check: gt_perm[b] == eye * (i < src_ns[b]), all entries in {0,1}."""
    if gt_perm.shape != (B, N, M):
        return False
    if gt_perm.min() < 0.0:
        return False
    i = np.arange(N)
    rowmask = (i[None, :] < src_ns[:, None]).astype(np.float32)  # [B, N]
    d = gt_perm[:, i, i]
    if not np.array_equal(d, rowmask):
        return False
    if not np.array_equal(gt_perm.sum(axis=2), rowmask):
        return False
    return True


def _reference_numpy(pred_dsmat, gt_perm, src_ns, tgt_ns, beta_value):
    """Direct numpy port of the reference — correctness fallback only."""
    out = 0.0
    n_sum = float(src_ns.astype(np.int64).sum())
    for b in range(pred_dsmat.shape[0]):
        p = pred_dsmat[b].astype(np.float64)
        g = gt_perm[b].astype(np.float64)
        s, t = int(src_ns[b]), int(tgt_ns[b])
        NN, MM = p.shape
        rm = (np.arange(NN) < s)
        cm = (np.arange(MM) < t)
        mask = rm[:, None] & cm[None, :]
        pred = np.clip(p, 0.0, 1.0) * mask
        gt = g * mask
        gp = pred * gt
        row_gt = gp.sum(1); col_gt = gp.sum(0)
        row_cnt = gt.sum(1); col_cnt = gt.sum(0)
        att_src = ((pred >= row_gt[:, None] - beta_value) & mask) * row_cnt[:, None]
        att_tgt = ((pred >= col_gt[None, :] - beta_value) & mask) * col_cnt[None, :]
        src_neg = (((att_src - gt) * pred) ** 2).sum(1)
        src_pos = (gp ** 2).sum(1)
        tgt_neg = (((att_tgt - gt) * pred) ** 2).sum(0)
        corr = (tgt_neg * col_cnt).sum()
        num = np.where(rm, src_pos, 1.0)
        den = np.where(rm, 1.0 + src_neg + corr, 1.0)
        out += -0.5 * (np.log(num / den) * rm).sum()
    return np.float32(out / n_sum)


def _host_prep(pred_dsmat, src_ns, tgt_ns, beta):
    ii = np.arange(N)
    rmask = (ii[None, :] < src_ns[:, None]).astype(np.float32)      # [B, N]
    diag = pred_dsmat[:, ii, ii].astype(np.float32)
    rowgt = np.clip(diag, 0.0, 1.0) * rmask                         # f32, exact
    srcpos = rowgt * rowgt
    thr = np.maximum(rowgt - np.float32(beta), 0.0)                 # clamped
    thr2 = (thr * thr).astype(np.float32)                           # [B, N]
    tau2 = thr2[:, :M].copy()
    tau2[:, NCOL:] = SENT                                           # sentinel cols
    tau2 = tau2.astype(ml_dtypes.float8_e4m3)

    pc = np.clip(pred_dsmat[:, :NR, :], 0.0, 1.0)
    q8 = (pc * pc).astype(ml_dtypes.float8_e4m3)                    # [B, NR, M]
    for gb in range(B):
        q8[gb, :, int(tgt_ns[gb]):] = 0                             # ragged col pad
        q8[gb, int(src_ns[gb]):, :] = 0                             # ragged row pad
        q8[gb, :, M - 1] = 0                                        # row-sum slot
    return rmask, srcpos, thr2, tau2, q8


def _make_in_maps(q8, thr2, tau2):
    in_maps = []
    for c in range(NCORES):
        b0 = c * BPC
        in_maps.append({
            "q8": np.ascontiguousarray(q8[b0:b0 + BPC]),
            "thr2": np.ascontiguousarray(thr2[b0:b0 + BPC, :NR]),
            "tau2": np.ascontiguousarray(tau2[b0:b0 + BPC]),
        })
    return in_maps


def _host_epilogue(t1row, t1col, srcpos, rmask, src_ns):
    """O(B*N) scalar epilogue on the device-computed threshold sums."""
    t1row = t1row.astype(np.float64)
    t1col = t1col.astype(np.float64)
    srcpos = srcpos.astype(np.float64)
    rmask = rmask.astype(np.float64)
    corr = ((t1col - srcpos) * rmask).sum(axis=1)                   # [B]
    src_neg = t1row - srcpos
    num = np.where(rmask > 0, np.maximum(srcpos, 1e-300), 1.0)
    den = np.where(rmask > 0, 1.0 + src_neg + corr[:, None], 1.0)
    total = -0.5 * (np.log(num / den) * rmask).sum()
    n_sum = float(src_ns.astype(np.int64).sum())
    return np.float32(total / n_sum)


def kernel(pred_dsmat, gt_perm, src_ns, tgt_ns, beta_value):
    pred_dsmat = np.asarray(pred_dsmat, dtype=np.float32)
    gt_perm = np.asarray(gt_perm, dtype=np.float32)
    src_ns = np.asarray(src_ns, dtype=np.int32)
    tgt_ns = np.asarray(tgt_ns, dtype=np.int32)
    beta = float(np.asarray(beta_value))

    if not _gt_is_identity_perm(gt_perm, src_ns) or int(src_ns.max()) > NR:
        return _reference_numpy(pred_dsmat, gt_perm, src_ns, tgt_ns, beta)

    from concourse.bass_utils import run_bass_kernel_spmd

    nc = _get_program()
    rmask, srcpos, thr2, tau2, q8 = _host_prep(pred_dsmat, src_ns, tgt_ns, beta)
    in_maps = _make_in_maps(q8, thr2, tau2)
    res = run_bass_kernel_spmd(nc, in_maps, list(range(NCORES)))
    t1row_c = np.concatenate([r["t1row"] for r in res.results], axis=0)  # [B, PT, CHR]
    t1row = np.zeros((B, N), np.float32)
    # device layout is [p, k]; full row index is i = k*PT + p
    t1row[:, :NR] = t1row_c.astype(np.float32).transpose(0, 2, 1).reshape(B, NR)
    t1col = np.zeros((B, N), np.float32)
    t1col[:, :NCOL] = np.concatenate(
        [r["t1col"] for r in res.results], axis=0
    ).astype(np.float32)
    return _host_epilogue(t1row, t1col, srcpos, rmask, src_ns)


# revision 7
# speedup vs baseline: 1.1388x; 1.1388x over previous
"""Trainium2 Bass kernel for nn_ContrastiveLossWithAttention.

Contract: kernel(**inputs) takes the FULL unsharded inputs (as produced by
reference.setup_inputs) and returns the FULL output (a float32 scalar).

Sharding: pure data parallel — batch dim B=16 split as 2 batches per core
across 8 NeuronCores. Host applies the O(B*N) scalar epilogue and the final
scalar reduction across cores.

Algorithm (gt_perm is an identity permutation restricted to rows i < src_ns,
verified exactly host-side with a numpy fallback): the loss collapses to two
masked threshold reductions over pred alone:
    T1row[i] = sum_j 1{p_ij >= t_i} p_ij^2      t_i = clip(diag_i) - beta
    T1col[j] = sum_i 1{p_ij >= tau_j} p_ij^2    tau_j = same vector, j-indexed
    corr     = sum_{j<s} (T1col_j - srcpos_j)
    loss_b   = -0.5 sum_{i<s} [ln(srcpos_i) - ln(1 + T1row_i - srcpos_i + corr)]

Since p >= 0 and thresholds are clamped at 0, p >= t  <=>  p^2 >= max(t,0)^2.
The host ships q = fp8_e4m3(clip(p)^2) (same role as the baseline's bf16 cast,
half the bytes) plus per-row/per-col squared thresholds. The device then needs
a single fused custom-DVE pass per 128-row chunk (registered at import as
CONTRA_FUSED_ANT, 1x mode):
    out[k]  = (q[k] >= tau2[k]) q[k]  +  (tau2[k] < 0) * cumsum((q[k] >= t2) q[k])
With tau2 = -128 for j >= 1536 (columns the epilogue never reads; src_ns <=
1536) and q[:, 2047] host-zeroed, out[:, :1536] is the column product that PE
ones-matmuls accumulate into T1col, and out[:, 2047] is the full row sum
T1row for the chunk (scalar-engine copy harvests it). One DVE instruction per
chunk replaces the baseline's STT + 2 TT + ACT-square pipeline.
"""

import numpy as np
import ml_dtypes

B, N, M = 16, 2048, 2048
NCORES = 8
BPC = B // NCORES      # batches per core
PT = 128               # partitions
CHR = 12               # row chunks computed: src_ns <= 1536 always
                       # (setup_inputs range); guarded with a fallback
NR = PT * CHR          # rows computed on device
NCOL = 1536            # columns with real col-thresholds (j < src max)
NQ = NCOL // 512       # 512-wide PSUM quadrants for PE column sums
SENT = -128.0          # sentinel tau2 for j >= NCOL (negative => gate on)

_cache = {}


def _register_op():
    import concourse.dve_ops as dve_ops
    from concourse.dve_spec import Spec, Src0, Src1, C0, Zero, AluOp, scan, lower
    from concourse.dve_uop import DveOpSpec
    from concourse.dve_table_gen import dve_ver_for

    for op in dve_ops.OPS:
        if op.name == "CONTRA_FUSED_ANT":
            return op

    cmp_c = Src0 >= Src1
    mul_c = cmp_c * Src0
    cmp_r = Src0 >= C0
    mul_r = cmp_r * Src0
    R = scan(AluOp.ADD, mul_r)
    gate = Src1 < Zero
    body = mul_c + gate * R

    def _ref(in0, in1, c0, c1, c2):
        f = in0.astype(np.float32)
        t = in1.astype(np.float32)
        colp = (f >= t).astype(np.float32) * f
        rowp = (f >= c0).astype(np.float32) * f
        Rv = np.cumsum(rowp, axis=-1, dtype=np.float32)
        return colp + (t < 0).astype(np.float32) * Rv

    spec = Spec(body=body, reference=_ref)
    ver = dve_ver_for("TRN2")
    row = max(dve_ops._SUB_OPCODE_FOR_NAME.values()) + 1
    sha = DveOpSpec(name="CONTRA_FUSED_ANT", opcode=row,
                    uops=lower(spec, ver=ver), rd1_en=True).sha(ver)
    op = dve_ops.DveOp("CONTRA_FUSED_ANT", spec, subdim=False,
                       uops_sha={ver: sha})
    dve_ops.OPS.append(op)
    dve_ops.CUSTOM_DVE_SPECS[op.name] = op.spec
    dve_ops._SUB_OPCODE_FOR_NAME[op.name] = row
    return op


def _build_program():
    import concourse.tile as tile
    from concourse import bacc, mybir

    OP = _register_op()
    f32 = mybir.dt.float32
    bf16 = mybir.dt.bfloat16
    fp8 = mybir.dt.float8e4

    nc = bacc.Bacc("TRN2", debug=False, num_devices=NCORES)

    q_d = nc.dram_tensor("q8", [BPC, NR, M], fp8, kind="ExternalInput")
    thr_d = nc.dram_tensor("thr2", [BPC, NR], f32, kind="ExternalInput")
    tau_d = nc.dram_tensor("tau2", [BPC, M], bf16, kind="ExternalInput")
    # t1row is [p, k] (partition-major) so the output DMA is contiguous per
    # partition; the host untransposes. A "(k p)" layout makes the DMA emit
    # 1536 4-byte descriptors (~10us exposed tail).
    t1r_d = nc.dram_tensor("t1row", [BPC, PT, CHR], f32, kind="ExternalOutput")
    t1c_d = nc.dram_tensor("t1col", [BPC, NCOL], f32, kind="ExternalOutput")

    with tile.TileContext(nc) as tc:
        with (
            tc.tile_pool(name="consts", bufs=1) as consts,
            tc.tile_pool(name="pb", bufs=2) as pb,
            tc.tile_pool(name="io", bufs=6) as io,
            tc.tile_pool(name="work", bufs=8) as work,
            tc.tile_pool(name="ps_col", bufs=2, space="PSUM") as ps_col,
        ):
            ones16 = consts.tile([PT, 1], bf16, tag="ones16")
            nc.vector.memset(ones16, 1.0)

            # prefetch all per-batch vectors before the chunk loops so the
            # batch-1 thresholds are ready the moment batch 0 drains
            thr2s, tau2s = [], []
            for b in range(BPC):
                thr2 = pb.tile([PT, CHR], f32, tag="thr2")
                nc.sync.dma_start(out=thr2, in_=thr_d[b].rearrange("(k p) -> p k", p=PT))
                tau2 = pb.tile([PT, M], bf16, tag="tau2")
                nc.sync.dma_start(
                    out=tau2, in_=tau_d[b:b + 1, :].to_broadcast([PT, M])
                )
                thr2s.append(thr2)
                tau2s.append(tau2)

            for b in range(BPC):
                thr2, tau2 = thr2s[b], tau2s[b]
                t1c_ps = ps_col.tile([1, NCOL], f32, tag="t1col")
                t1row = pb.tile([PT, CHR], f32, tag="t1row")
                for k in range(CHR):
                    q = io.tile([PT, M], fp8, tag="q")
                    nc.sync.dma_start(out=q, in_=q_d[b, k * PT:(k + 1) * PT, :])
                    oc = work.tile([PT, M], bf16, tag="oc")
                    nc.vector._custom_dve(
                        OP, out=oc, in0=q, in1=tau2,
                        s0=thr2[:, k:k + 1], s1=0.0,
                    )
                    nc.scalar.copy(t1row[:, k:k + 1], oc[:, M - 1:M])
                    for qd in range(NQ):
                        nc.tensor.matmul(
                            t1c_ps[0:1, qd * 512:(qd + 1) * 512],
                            ones16,
                            oc[:, qd * 512:(qd + 1) * 512],
                            start=(k == 0), stop=(k == CHR - 1),
                        )

                t1c_row = pb.tile([1, NCOL], f32, tag="t1c_row")
                nc.scalar.copy(t1c_row, t1c_ps[0:1, :])
                nc.sync.dma_start(out=t1c_d[b:b + 1, :], in_=t1c_row)
                nc.sync.dma_start(out=t1r_d[b], in_=t1row)

    nc.compile()
    return nc


def _get_program():
    if "nc" not in _cache:
        _cache["nc"] = _build_program()
    return _cache["nc"]


def _gt_is_identity_perm(gt_perm, src_ns):
    """Exact check: gt_perm[b] == eye * (i < src_ns[b]), all entries in {0,1}."""
    if gt_perm.shape != (B, N, M):
        return False
    if gt_perm.min() < 0.0:
        return False
    i = np.arange(N)
    rowmask = (i[None, :] < src_ns[:, None]).astype(np.float32)  # [B, N]
    d = gt_perm[:, i, i]
    if not np.array_equal(d, rowmask):
        return False
    if not np.array_equal(gt_perm.sum(axis=2), rowmask):
        return False
    return True


def _reference_numpy(pred_dsmat, gt_perm, src_ns, tgt_ns, beta_value):
    """Direct numpy port of the reference — correctness fallback only."""
    out = 0.0
    n_sum = float(src_ns.astype(np.int64).sum())
    for b in range(pred_dsmat.shape[0]):
        p = pred_dsmat[b].astype(np.float64)
        g = gt_perm[b].astype(np.float64)
        s, t = int(src_ns[b]), int(tgt_ns[b])
        NN, MM = p.shape
        rm = (np.arange(NN) < s)
        cm = (np.arange(MM) < t)
        mask = rm[:, None] & cm[None, :]
        pred = np.clip(p, 0.0, 1.0) * mask
        gt = g * mask
        gp = pred * gt
        row_gt = gp.sum(1); col_gt = gp.sum(0)
        row_cnt = gt.sum(1); col_cnt = gt.sum(0)
        att_src = ((pred >= row_gt[:, None] - beta_value) & mask) * row_cnt[:, None]
        att_tgt = ((pred >= col_gt[None, :] - beta_value) & mask) * col_cnt[None, :]
        src_neg = (((att_src - gt) * pred) ** 2).sum(1)
        src_pos = (gp ** 2).sum(1)
        tgt_neg = (((att_tgt - gt) * pred) ** 2).sum(0)
        corr = (tgt_neg * col_cnt).sum()
        num = np.where(rm, src_pos, 1.0)
        den = np.where(rm, 1.0 + src_neg + corr, 1.0)
        out += -0.5 * (np.log(num / den) * rm).sum()
    return np.float32(out / n_sum)


def _host_prep(pred_dsmat, src_ns, tgt_ns, beta):
    ii = np.arange(N)
    rmask = (ii[None, :] < src_ns[:, None]).astype(np.float32)      # [B, N]
    diag = pred_dsmat[:, ii, ii].astype(np.float32)
    rowgt = np.clip(diag, 0.0, 1.0) * rmask                         # f32, exact
    srcpos = rowgt * rowgt
    thr = np.maximum(rowgt - np.float32(beta), 0.0)                 # clamped
    thr2 = (thr * thr).astype(np.float32)                           # [B, N]
    tau2 = thr2[:, :M].copy()
    tau2[:, NCOL:] = SENT                                           # sentinel cols
    tau2 = tau2.astype(ml_dtypes.bfloat16)

    pc = np.clip(pred_dsmat[:, :NR, :], 0.0, 1.0)
    q8 = (pc * pc).astype(ml_dtypes.float8_e4m3)                    # [B, NR, M]
    for gb in range(B):
        q8[gb, :, int(tgt_ns[gb]):] = 0                             # ragged col pad
        q8[gb, int(src_ns[gb]):, :] = 0                             # ragged row pad
        q8[gb, :, M - 1] = 0                                        # row-sum slot
    return rmask, srcpos, thr2, tau2, q8


def _make_in_maps(q8, thr2, tau2):
    in_maps = []
    for c in range(NCORES):
        b0 = c * BPC
        in_maps.append({
            "q8": np.ascontiguousarray(q8[b0:b0 + BPC]),
            "thr2": np.ascontiguousarray(thr2[b0:b0 + BPC, :NR]),
            "tau2": np.ascontiguousarray(tau2[b0:b0 + BPC]),
        })
    return in_maps


def _host_epilogue(t1row, t1col, srcpos, rmask, src_ns):
    """O(B*N) scalar epilogue on the device-computed threshold sums."""
    t1row = t1row.astype(np.float64)
    t1col = t1col.astype(np.float64)
    srcpos = srcpos.astype(np.float64)
    rmask = rmask.astype(np.float64)
    corr = ((t1col - srcpos) * rmask).sum(axis=1)                   # [B]
    src_neg = t1row - srcpos
    num = np.where(rmask > 0, np.maximum(srcpos, 1e-300), 1.0)
    den = np.where(rmask > 0, 1.0 + src_neg + corr[:, None], 1.0)
    total = -0.5 * (np.log(num / den) * rmask).sum()
    n_sum = float(src_ns.astype(np.int64).sum())
    return np.float32(total / n_sum)


def kernel(pred_dsmat, gt_perm, src_ns, tgt_ns, beta_value):
    pred_dsmat = np.asarray(pred_dsmat, dtype=np.float32)
    gt_perm = np.asarray(gt_perm, dtype=np.float32)
    src_ns = np.asarray(src_ns, dtype=np.int32)
    tgt_ns = np.asarray(tgt_ns, dtype=np.int32)
    beta = float(np.asarray(beta_value))

    if not _gt_is_identity_perm(gt_perm, src_ns) or int(src_ns.max()) > NR:
        return _reference_numpy(pred_dsmat, gt_perm, src_ns, tgt_ns, beta)

    from concourse.bass_utils import run_bass_kernel_spmd

    nc = _get_program()
    rmask, srcpos, thr2, tau2, q8 = _host_prep(pred_dsmat, src_ns, tgt_ns, beta)
    in_maps = _make_in_maps(q8, thr2, tau2)
    res = run_bass_kernel_spmd(nc, in_maps, list(range(NCORES)))
    t1row_c = np.concatenate([r["t1row"] for r in res.results], axis=0)  # [B, PT, CHR]
    t1row = np.zeros((B, N), np.float32)
    # device layout is [p, k]; full row index is i = k*PT + p
    t1row[:, :NR] = t1row_c.astype(np.float32).transpose(0, 2, 1).reshape(B, NR)
    t1col = np.zeros((B, N), np.float32)
    t1col[:, :NCOL] = np.concatenate(
        [r["t1col"] for r in res.results], axis=0
    ).astype(np.float32)
    return _host_epilogue(t1row, t1col, srcpos, rmask, src_ns)
